# revision 59
# baseline (speedup 1.0000x reference)
"""Trainium2 Bass kernel for an 8x(2048,32) decoder block.

Sharding: data-parallel over batch. B=8 batch elements -> 8 NeuronCores,
one batch element per core, parameters replicated, no collectives.

Layouts (t = 512*g + 128*j + p;  n = 4*g + j;  g,j in [0,4), p in [0,128)):
  row-major ("rm"):  tile[p, n, d]
  chunk-transposed ("ct"): tile[32*j + d, (g, p)]
  Q^T/K^T: tile[32*h + hd, (j, g, p)] bf16 (8 valid rows per 32-row block)

Attention (head dim 8, 4 heads):
  S^T[kv, q] per (kv-chunk c, q-group g) via K=8 matmuls, two PSUM tiles of
  2 heads each (double-buffered) so PE can run chunk c+1's S while ScalarE
  still exps chunk c (software-pipelined emission; O matmuls for chunk c
  are emitted after chunk c+1's S matmuls).
  P = exp(S^T): full (off-diagonal) chunks exp to fp8-e4m3 pairs, consumed
  by DoubleRow fp8 matmuls (2 kv-chunks per matmul, 0.5 cyc/col); diagonal
  chunks exp to bf16, triangular mask multiply on VectorE, bf16 matmuls.
  Some full pairs are exp'd on VectorE instead via the Schraudolph int-bias
  trick (x*A+B converted to int8/int16, bit-viewed as fp8/bf16) to offload
  the saturated ScalarE; the ~3% piecewise-linear ripple is harmless here
  (softmax normalization cancels the uniform bias; o' is a small residual
  contribution).
  O'^T[(dout|den), q] += [16*V'_h | 16].T accumulation in PSUM where
  V'_h = Wv[h] @ Wproj rows (folded host-side); the x16 scale puts V' in
  fp8's normal range and cancels in the softmax division.
  Per-g tail: transpose O' back, fused divide/head-sum/residual, LN2 with
  VectorE Newton rsqrt (ScalarE never swaps activation tables), FFN, and
  the output DMA — all pipelined under the next group's attention.
"""

import ml_dtypes
import numpy as np

import concourse.bacc as bacc
import concourse.bass as bass
import concourse.mybir as mybir
import concourse.tile as tile
from concourse.bass import ts
from concourse.bass_utils import run_bass_kernel_spmd

B, T, D, H, HD = 8, 2048, 32, 4, 8
P = 128
NCORES = 8
FF = 4 * D  # 128
FP32 = mybir.dt.float32
BF16 = mybir.dt.bfloat16
FP8 = mybir.dt.float8e4
I8 = mybir.dt.int8
I16 = mybir.dt.int16
I32 = mybir.dt.int32
AF = mybir.ActivationFunctionType
ALU = mybir.AluOpType
AX = mybir.AxisListType
DR = mybir.MatmulPerfMode.DoubleRow
EPS = 1e-5
VSCALE = 16.0

USE_FP8_DR = True
DVE_PAIRS = False
import os
KBISECT = int(os.environ.get("KBISECT", "0"))  # 0=full; 1..4 truncated builds
# Schraudolph exp constants: y = bits(int(x * SCALE + BIAS))
EXP8_SCALE, EXP8_BIAS = 8.0 / np.log(2.0), 56.0 - 0.344
EXP16_SCALE, EXP16_BIAS = 128.0 / np.log(2.0), 16256.0 - 5.5


def _dve_pair(g, pi):
    """Full-chunk pairs routed to VectorE exp (ScalarE offload)."""
    return DVE_PAIRS and (g, pi) in {(2, 3), (3, 4)}


# fp32 blob column offsets (params first so one contiguous DMA covers the
# params + x block g0 that LN1(g0) needs)
_G1 = 0            # ln1_g rep [32]
_B1 = 32           # ln1_b rep [32]
_G2 = 64           # ln2_g rep [32]
_LB2 = 96          # ln2_b rep [32]
_BPJ = 128         # bproj rep [32]
_B2R = 160         # ffn b2 rep [32]
_FB1 = 192         # ffn b1 per-partition [1]
_B2C = 193         # ffn b2 in ct layout (per-partition 32j+d -> b2[d]) [1]
_X0 = 194          # x_rm [512]
NF32 = 706

# bf16 blob column offsets
_WQ = 0            # wq_pad [128]
_WK = 128          # wk_pad [128]
_WVP = 256         # wvp [128]
_W1 = 384          # w1 tiled [128]
_W2 = 512          # w2 [32]
_IDB = 544         # identity [128]
_MSK = 672         # causal mask [128]
NBF = 800

_NC_CACHE = {}


def _build_nc():
    nc = bacc.Bacc(
        "TRN2",
        target_bir_lowering=False,
        debug=False,
        enable_asserts=False,
        num_devices=NCORES,
    )
    bf_d = nc.dram_tensor("bblob", [P, NBF], BF16, kind="ExternalInput").ap()
    f32_d = nc.dram_tensor("fblob", [P, NF32], FP32, kind="ExternalInput").ap()
    y_d = nc.dram_tensor("y", [P, 512], FP32, kind="ExternalOutput").ap()

    with tile.TileContext(nc) as tc:
        _decoder_body(tc, f32_d, bf_d, y_d)
    nc.compile()
    return nc


def _decoder_body(tc, f32_d, bf_d, y_d):
    nc = tc.nc

    with (
        tc.tile_pool(name="pers", bufs=1) as pers,
        tc.tile_pool(name="work", bufs=2) as work,
        tc.tile_pool(name="ps", bufs=1, space="PSUM") as ps,
    ):
        fb = pers.tile([P, NF32], FP32)
        # params + x block g0 first so LN1(g0) starts ASAP; rest of x after.
        nc.sync.dma_start(fb[:, 0 : _X0 + 128], f32_d[:, 0 : _X0 + 128])
        bb = pers.tile([P, NBF], BF16)
        nc.sync.dma_start(bb[:], bf_d)
        nc.sync.dma_start(fb[:, _X0 + 128 : NF32], f32_d[:, _X0 + 128 : NF32])

        x3 = fb[:, _X0 : _X0 + 512].rearrange("p (n d) -> p n d", d=D)
        g1 = fb[:, _G1 : _G1 + D]
        b1 = fb[:, _B1 : _B1 + D]
        g2 = fb[:, _G2 : _G2 + D]
        lb2 = fb[:, _LB2 : _LB2 + D]
        bpj = fb[:, _BPJ : _BPJ + D]
        b2r = fb[:, _B2R : _B2R + D]
        fb1 = fb[:, _FB1 : _FB1 + 1]
        b2c = fb[:, _B2C : _B2C + 1]
        wq = bb[:, _WQ : _WQ + P]
        wk = bb[:, _WK : _WK + P]
        wvp = bb[:, _WVP : _WVP + P]
        w1 = bb[:, _W1 : _W1 + P]
        w2 = bb[:, _W2 : _W2 + D]
        idb = bb[:, _IDB : _IDB + P]
        msk = bb[:, _MSK : _MSK + P]

        def rsqrt(out_ap, in_ap, shape, tag):
            """out = 1/sqrt(in) via bit-hack + 2 Newton steps (all VectorE)."""
            y0 = work.tile(shape, FP32, tag=tag + "y0", name=tag + "y0")
            sh = work.tile(shape, I32, tag=tag + "sh", name=tag + "sh")
            nc.vector.tensor_scalar(
                sh[:], in_ap.bitcast(I32), 1, None, op0=ALU.logical_shift_right,
            )
            nc.vector.tensor_scalar(
                y0.bitcast(I32), sh[:], -1, 0x5F3759DF, op0=ALU.mult, op1=ALU.add,
            )
            a = work.tile(shape, FP32, tag=tag + "a", name=tag + "a")
            for it in range(2):
                nc.vector.tensor_mul(a[:], y0[:], y0[:])
                nc.vector.tensor_mul(a[:], a[:], in_ap)
                nc.vector.tensor_scalar(a[:], a[:], -0.5, 1.5, op0=ALU.mult, op1=ALU.add)
                nc.vector.tensor_mul(out_ap if it == 1 else y0[:], y0[:], a[:])

        def layer_norm(src3, g_ap, b_ap, out3, ngrp, tag, ew=None):
            """src3/out3: [P, ngrp, D]; per-(p,group) LN over d. `ew` picks the
            engine for the big elementwise ops (default VectorE)."""
            ew = ew or nc.vector
            mus = work.tile([P, ngrp], FP32, tag=tag + "mu", name=tag + "mu")
            nc.vector.reduce_sum(mus[:], src3, axis=AX.X)
            nc.vector.tensor_scalar(mus[:], mus[:], 1.0 / D, None, op0=ALU.mult)
            xc = work.tile([P, ngrp * D], FP32, tag=tag + "xc", name=tag + "xc")
            xc3 = xc.rearrange("p (n d) -> p n d", d=D)
            ew.tensor_sub(xc3, src3, mus[:, :, None].to_broadcast((P, ngrp, D)))
            sq = work.tile([P, ngrp * D], FP32, tag=tag + "sq", name=tag + "sq")
            sq3 = sq.rearrange("p (n d) -> p n d", d=D)
            ew.tensor_mul(sq3, xc3, xc3)
            vs = work.tile([P, ngrp], FP32, tag=tag + "vs", name=tag + "vs")
            nc.vector.reduce_sum(vs[:], sq3, axis=AX.X)
            nc.vector.tensor_scalar(vs[:], vs[:], 1.0 / D, EPS, op0=ALU.mult, op1=ALU.add)
            istd = work.tile([P, ngrp], FP32, tag=tag + "is", name=tag + "is")
            rsqrt(istd[:], vs[:], [P, ngrp], tag)
            ew.tensor_mul(xc3, xc3, istd[:, :, None].to_broadcast((P, ngrp, D)))
            ew.tensor_mul(xc3, xc3, g_ap[:, None, :].to_broadcast((P, ngrp, D)))
            ew.tensor_add(out3, xc3, b_ap[:, None, :].to_broadcast((P, ngrp, D)))

        h_rm = pers.tile([P, 512], FP32)
        h3 = h_rm.rearrange("p (n d) -> p n d", d=D)
        hb = pers.tile([P, 512], FP32)
        hb3 = hb.rearrange("p (n d) -> p n d", d=D)
        h_bf = pers.tile([P, 512], BF16)
        h_ct = pers.tile([P, 512], BF16)
        qt = pers.tile([P, T], BF16)
        kt = pers.tile([P, T], BF16)
        qt4 = qt.rearrange("p (j g q) -> p j g q", j=4, g=4)
        kt4 = kt.rearrange("p (j g q) -> p j g q", j=4, g=4)
        v8 = pers.tile([P, 16 * H * 64], FP8)
        v84 = v8.rearrange("p (c h e) -> p c h e", c=16, h=H)
        nc.gpsimd.memset(v8.bitcast(I32), 0)
        nc.vector.memset(v84[:, :, :, 32], VSCALE)
        vb4 = None
        if not USE_FP8_DR:
            vb = pers.tile([P, 16 * H * 64], BF16)
            vb4 = vb.rearrange("p (c h e) -> p c h e", c=16, h=H)
            nc.gpsimd.memset(vb.bitcast(I32), 0)
            nc.vector.memset(vb4[:, :, :, 32], VSCALE)

        x1_rm = pers.tile([P, 512], FP32)
        x13 = x1_rm.rearrange("p (n d) -> p n d", d=D)
        h2_rm = pers.tile([P, 512], FP32)
        h23 = h2_rm.rearrange("p (n d) -> p n d", d=D)
        h2_bf = pers.tile([P, 512], BF16)
        h2b3 = h2_bf.rearrange("p (n d) -> p n d", d=D)
        a_sb = pers.tile([FF, 16 * P], BF16)
        a_sb3 = a_sb.rearrange("f (n p) -> f n p", n=16)
        y_rm = pers.tile([P, 512], FP32)
        y3 = y_rm.rearrange("p (n d) -> p n d", d=D)

        def emit_ln1(n0, n1, tag, ew=None):
            layer_norm(x3[:, n0:n1, :], g1, b1, h3[:, n0:n1, :], n1 - n0, tag, ew=ew)
            nc.vector.tensor_add(
                hb3[:, n0:n1, :], h3[:, n0:n1, :],
                bpj[:, None, :].to_broadcast((P, n1 - n0, D)))

        def emit_hct(gs):
            nc.scalar.activation(
                h_bf[:, 128 * gs[0] : 128 * (gs[-1] + 1)],
                h_rm[:, 128 * gs[0] : 128 * (gs[-1] + 1)], AF.Copy)
            hct_ps = ps.tile([P, len(gs) * P], BF16, tag="t", name="hct_ps")
            for i, g in enumerate(gs):
                nc.tensor.transpose(hct_ps[:, ts(i, P)], h_bf[:, ts(g, P)], idb)
            nc.scalar.activation(
                h_ct[:, 128 * gs[0] : 128 * (gs[-1] + 1)], hct_ps[:], AF.Copy)

        def qkv_pieces(g):
            """Per-j emission pieces for Q^T/K^T cols (:, g, :) and V'
            chunks c=4g+j. Each matmul gets its own PSUM tile -- matmuls
            with different tile_position into one PSUM bank fault on HW."""
            def mk_q(j):
                def f():
                    q_ps = ps.tile([P, P], FP32, tag="t", name="q_ps")
                    nc.tensor.matmul(
                        q_ps[:], lhsT=wq[ts(j, 32), :],
                        rhs=h_ct[ts(j, 32), ts(g, P)],
                        start=True, stop=True, tile_position=(32 * j, 0))
                    if g <= 1:
                        nc.scalar.activation(qt4[:, j, g, :], q_ps[:], AF.Copy)
                    else:
                        nc.vector.tensor_copy(qt4[:, j, g, :], q_ps[:])
                return f

            def mk_k(j):
                def f():
                    k_ps = ps.tile([P, P], FP32, tag="t", name="k_ps")
                    nc.tensor.matmul(
                        k_ps[:], lhsT=wk[ts(j, 32), :],
                        rhs=h_ct[ts(j, 32), ts(g, P)],
                        start=True, stop=True, tile_position=(32 * j, 0))
                    if g <= 1:
                        nc.scalar.activation(kt4[:, j, g, :], k_ps[:], AF.Copy)
                    else:
                        nc.vector.tensor_copy(kt4[:, j, g, :], k_ps[:])
                return f

            def mk_v(j):
                def f():
                    c = 4 * g + j
                    vp_ps = ps.tile([P, P], FP32, tag="t", name="vp_ps")
                    nc.tensor.matmul(
                        vp_ps[:], lhsT=h_ct[ts(j, 32), ts(g, P)],
                        rhs=wvp[ts(j, 32), :],
                        start=True, stop=True, tile_position=(32 * j, 0))
                    vps = vp_ps.rearrange("p (h e) -> p h e", e=32)
                    if g <= 1:
                        nc.scalar.activation(v84[:, c, :, 0:32], vps, AF.Copy,
                                             scale=VSCALE)
                    else:
                        nc.vector.tensor_scalar(
                            v84[:, c, :, 0:32], vps, VSCALE, None, op0=ALU.mult)
                return f

            out = [mk_q(j) for j in range(4)] + [mk_k(j) for j in range(4)]
            if KBISECT != 15:
                out += [mk_v(j) for j in range(4)]
            return out

        def emit_qkv(g):
            for f in qkv_pieces(g):
                f()
                if not USE_FP8_DR:
                    nc.vector.tensor_scalar(
                        vb4[:, c, :, 0:32], vps, VSCALE, None, op0=ALU.mult)

        def emit_ffn(gg):
            """h2 block gg -> y block gg -> DMA out."""
            h2t = ps.tile([P, P], BF16, tag="t", name="h2t")
            nc.tensor.transpose(h2t[:], h2_bf[:, ts(gg, P)], idb)
            h2c = work.tile([P, P], BF16, tag="h2c", name="h2c")
            nc.vector.tensor_copy(h2c[:], h2t[:])
            for j in range(4):
                a_ps = ps.tile([P, P], FP32, tag="t", name="a_ps")
                nc.tensor.matmul(
                    a_ps[:], lhsT=w1[ts(j, 32), :], rhs=h2c[ts(j, 32), :],
                    start=True, stop=True, tile_position=(32 * j, 0))
                nc.vector.tensor_scalar(
                    a_sb3[:, 4 * gg + j, :], a_ps[:],
                    fb1, 0.0, op0=ALU.add, op1=ALU.max)
            ff_ps = ps.tile([P, P], FP32, tag="t", name="ff_ps")
            for j in range(4):
                nc.tensor.matmul(
                    ff_ps[ts(j, 32), :], lhsT=w2, rhs=a_sb3[:, 4 * gg + j, :],
                    start=True, stop=True, tile_position=(0, 32 * j))
            ffb = work.tile([P, P], BF16, tag="ffb", name="ffb")
            nc.vector.tensor_scalar(ffb[:], ff_ps[:], b2c, None, op0=ALU.add)
            ftp = ps.tile([P, P], BF16, tag="t", name="ftp")
            nc.tensor.transpose(ftp[:], ffb[:], idb)
            yb = y3[:, 4 * gg : 4 * gg + 4, :]
            nc.vector.tensor_add(yb, ftp.rearrange("p (n d) -> p n d", d=D),
                                 h23[:, 4 * gg : 4 * gg + 4, :])
            nc.sync.dma_start(y_d[:, ts(gg, P)], y_rm[:, ts(gg, P)])

        def bisect_out(src_tile):
            nc.vector.tensor_copy(y_rm[:], src_tile[:])
            nc.sync.dma_start(y_d, y_rm[:])

        # ---- attention + per-g postprocessing, software-pipelined ----
        pend = []
        pend_tail = []
        for g in range(4):
            if g == 1:
                emit_hct([1, 2, 3])
                emit_qkv(1)
                emit_qkv(2)
            elif g == 2:
                emit_qkv(3)
            if g == 0:
                if KBISECT == 11:
                    nc.vector.tensor_copy(y_rm[:], fb[:, _X0 : _X0 + 512])
                    nc.sync.dma_start(y_d, y_rm[:])
                    return
                emit_ln1(0, 4, "lnA")
                if KBISECT == 12:
                    bisect_out(h_rm)
                    return
                emit_hct([0])
                if KBISECT == 13:
                    bisect_out(h_rm)
                    return
                emit_qkv(0)
                if KBISECT in (14, 15, 16):
                    bisect_out(h_rm)
                    return
                if KBISECT == 1:
                    emit_ln1(4, 16, "lnC")
                    bisect_out(h_rm)
                    return

            oA = ps.tile([P, 512], FP32, tag="o", bufs=2, name="oA")
            oB = ps.tile([P, 512], FP32, tag="o", bufs=2, name="oB")
            nch = 4 * g + 4
            p8_live = None
            for c in range(nch):
                m = c - 4 * g
                diag = m >= 0
                lo = 128 * m if diag else 0
                loj = lo // 128
                gc_, jc = divmod(c, 4)
                s0 = ps.tile([P, 2 * 512], FP32, tag="s", bufs=2, name="s0")
                s03 = s0.rearrange("p (h q) -> p h q", h=2)
                s1 = ps.tile([P, 2 * 512], FP32, tag="s", bufs=2, name="s1")
                s13 = s1.rearrange("p (h q) -> p h q", h=2)
                for h in range(4):
                    st = s03 if h < 2 else s13
                    nc.tensor.matmul(
                        st[:, h % 2, lo:],
                        lhsT=kt4[32 * h : 32 * h + HD, jc, gc_, :],
                        rhs=qt4[32 * h : 32 * h + HD, loj:, g, :],
                        start=True, stop=True,
                        tile_position=(32 * h, 0),
                    )
                for fn in pend:
                    fn()
                pend = []
                if diag and m == 2 and KBISECT != 3:
                    mk_half(g, oA, oB, 0)()
                    if g == 3:
                        mk_ln2h(g, 0)()
                if pend_tail:
                    pend_tail.pop(0)()
                if g == 0 and c == 1:
                    # LN1 for groups 1..3: scheduled behind g0's critical
                    # chain (the tile scheduler would otherwise interleave it
                    # into LN1-g0 on VectorE and delay the whole pipeline),
                    # with the big elementwise ops on the idle GpSimd engine.
                    with tc.tile_wait_until(0.009):
                        emit_ln1(4, 16, "lnB", ew=nc.gpsimd)
                if diag:
                    pd = work.tile([P, 4 * 512], BF16, tag="pd", bufs=4, name="pd")
                    pd3 = pd.rearrange("p (h q) -> p h q", h=4)
                    nc.scalar.activation(pd3[:, 0:2, lo:], s03[:, :, lo:], AF.Exp)
                    nc.scalar.activation(pd3[:, 2:4, lo:], s13[:, :, lo:], AF.Exp)
                    nc.vector.tensor_mul(
                        pd3[:, :, lo : lo + P],
                        pd3[:, :, lo : lo + P],
                        msk[:, None, :].to_broadcast((P, 4, P)),
                    )

                    def mk_diag(c=c, lo=lo, pd3=pd3, oA=oA, oB=oB, nch=nch):
                        def f():
                            vsrc = v84 if USE_FP8_DR else vb4
                            for h in range(4):
                                ob = oA if h < 2 else oB
                                base = 64 * (h % 2)
                                nc.tensor.matmul(
                                    ob[base : base + 64, lo:],
                                    lhsT=vsrc[:, c, h, :],
                                    rhs=pd3[:, h, lo:],
                                    start=(c == 0),
                                    stop=(c == nch - 1),
                                    skip_group_check=True,
                                )
                        return f

                    pend.append(mk_diag())
                elif USE_FP8_DR:
                    par = c & 1
                    if par == 0:
                        p8_live = work.tile([P, 2 * 4 * 512], FP8, tag="p8",
                                            bufs=3, name="p8")
                    p84 = p8_live.rearrange("p (r h q) -> p r h q", r=2, h=4)
                    if _dve_pair(g, c // 2):
                        nc.vector.tensor_scalar(
                            p84.bitcast(I8)[:, par, 0:2, :], s03,
                            EXP8_SCALE, EXP8_BIAS, op0=ALU.mult, op1=ALU.add)
                        nc.vector.tensor_scalar(
                            p84.bitcast(I8)[:, par, 2:4, :], s13,
                            EXP8_SCALE, EXP8_BIAS, op0=ALU.mult, op1=ALU.add)
                    else:
                        nc.scalar.activation(p84[:, par, 0:2, :], s03, AF.Exp)
                        nc.scalar.activation(p84[:, par, 2:4, :], s13, AF.Exp)
                    if par == 1:
                        def mk_pair(c=c, p84=p84, oA=oA, oB=oB):
                            def f():
                                # even heads: DoubleRow fp8 (tile position 0
                                # only -- walrus rejects DR at col offset 64);
                                # odd heads: plain fp8 matmuls per chunk.
                                for h in (0, 2):
                                    ob = oA if h < 2 else oB
                                    for qh in range(2):
                                        nc.tensor.matmul(
                                            ob[0:64, ts(qh, 256)],
                                            lhsT=v84[:, c - 1 : c + 1, h, :],
                                            rhs=p84[:, :, h, ts(qh, 256)],
                                            perf_mode=DR,
                                            start=(c == 1),
                                            stop=False,
                                            skip_group_check=True,
                                        )
                                for h in (1, 3):
                                    ob = oA if h < 2 else oB
                                    for cc in (c - 1, c):
                                        nc.tensor.matmul(
                                            ob[64:128, :],
                                            lhsT=v84[:, cc, h, :],
                                            rhs=p84[:, cc & 1, h, :],
                                            start=(cc == 0),
                                            stop=False,
                                            tile_position=(0, 64),
                                            skip_group_check=True,
                                        )
                            return f

                        pend.append(mk_pair())
                else:
                    pdf = work.tile([P, 4 * 512], BF16, tag="pd", bufs=4, name="pdf")
                    pdf3 = pdf.rearrange("p (h q) -> p h q", h=4)
                    nc.scalar.activation(pdf3[:, 0:2, :], s03, AF.Exp)
                    nc.scalar.activation(pdf3[:, 2:4, :], s13, AF.Exp)

                    def mk_full(c=c, pdf3=pdf3, oA=oA, oB=oB):
                        def f():
                            for h in range(4):
                                ob = oA if h < 2 else oB
                                base = 64 * (h % 2)
                                nc.tensor.matmul(
                                    ob[base : base + 64, :],
                                    lhsT=vb4[:, c, h, :],
                                    rhs=pdf3[:, h, :],
                                    start=(c == 0),
                                    stop=False,
                                    skip_group_check=True,
                                )
                        return f

                    pend.append(mk_full())
            for fn in pend:
                fn()
            pend = []

            def mk_half(g, oA, oB, half):
                def f():
                    # finalize q'-columns [256*half, 256*half+256) of O':
                    # transpose back, divide by den, sum heads, residual.
                    osbA = work.tile([P, 256], BF16, tag="osb", bufs=4,
                                     name="osbA")
                    nc.vector.tensor_copy(osbA[:], oA[:, ts(half, 256)])
                    osbB = work.tile([P, 256], BF16, tag="osb", bufs=4,
                                     name="osbB")
                    nc.vector.tensor_copy(osbB[:], oB[:, ts(half, 256)])
                    otpA = ps.tile([P, 256], BF16, tag="t", name="otpA")
                    otpB = ps.tile([P, 256], BF16, tag="t", name="otpB")
                    for jj in range(2):
                        nc.tensor.transpose(otpA[:, ts(jj, P)],
                                            osbA[:, ts(jj, P)], idb)
                    for jj in range(2):
                        nc.tensor.transpose(otpB[:, ts(jj, P)],
                                            osbB[:, ts(jj, P)], idb)
                    oa4 = otpA.rearrange("p (j k e) -> p j k e", j=2, k=2)
                    ob4 = otpB.rearrange("p (j k e) -> p j k e", j=2, k=2)
                    drA = work.tile([P, 2 * 2], FP32, tag="dr", name="drA")
                    drA3 = drA.rearrange("p (j k) -> p j k", j=2)
                    nc.vector.reciprocal(drA3, oa4[:, :, :, 32])
                    drB = work.tile([P, 2 * 2], FP32, tag="dr", name="drB")
                    drB3 = drB.rearrange("p (j k) -> p j k", j=2)
                    nc.vector.reciprocal(drB3, ob4[:, :, :, 32])
                    tmA = work.tile([P, 2 * 2 * 32], FP32, tag="tm", name="tmA")
                    tmA4 = tmA.rearrange("p (j k e) -> p j k e", j=2, k=2)
                    nc.vector.tensor_mul(
                        tmA4, oa4[:, :, :, 0:32],
                        drA3[:, :, :, None].to_broadcast((P, 2, 2, 32)))
                    tmB = work.tile([P, 2 * 2 * 32], FP32, tag="tm", name="tmB")
                    tmB4 = tmB.rearrange("p (j k e) -> p j k e", j=2, k=2)
                    nc.vector.tensor_mul(
                        tmB4, ob4[:, :, :, 0:32],
                        drB3[:, :, :, None].to_broadcast((P, 2, 2, 32)))
                    n0 = 4 * g + 2 * half
                    u1 = work.tile([P, 2 * 32], FP32, tag="u", name="u1")
                    u13 = u1.rearrange("p (j e) -> p j e", j=2)
                    nc.vector.tensor_add(u13, tmA4[:, :, 0, :], tmA4[:, :, 1, :])
                    u2 = work.tile([P, 2 * 32], FP32, tag="u", name="u2")
                    u23 = u2.rearrange("p (j e) -> p j e", j=2)
                    nc.vector.tensor_add(u23, tmB4[:, :, 0, :], tmB4[:, :, 1, :])
                    nc.vector.tensor_add(u13, u13, u23)
                    nc.vector.tensor_add(x13[:, n0 : n0 + 2, :], u13,
                                         hb3[:, n0 : n0 + 2, :])
                return f

            def mk_ln2(g):
                def f():
                    layer_norm(x13[:, 4 * g : 4 * g + 4, :], g2, lb2,
                               h23[:, 4 * g : 4 * g + 4, :], 4, "ln2")
                    nc.vector.tensor_copy(h2b3[:, 4 * g : 4 * g + 4, :],
                                          h23[:, 4 * g : 4 * g + 4, :])
                return f

            def mk_ln2h(g, half):
                def f():
                    n0 = 4 * g + 2 * half
                    layer_norm(x13[:, n0 : n0 + 2, :], g2, lb2,
                               h23[:, n0 : n0 + 2, :], 2, "ln2")
                    nc.vector.tensor_copy(h2b3[:, n0 : n0 + 2, :],
                                          h23[:, n0 : n0 + 2, :])
                return f

            if KBISECT == 3:
                pend_tail = []
            elif KBISECT == 4:
                pend_tail = [mk_half(g, oA, oB, 1)]
                if g == 3:
                    pend_tail[0]()
                    pend_tail = []
            elif g == 3:
                mk_half(g, oA, oB, 1)()
                mk_ln2h(g, 1)()
                emit_ffn(g)
                pend_tail = []
            else:
                pend_tail = [mk_half(g, oA, oB, 1), mk_ln2(g),
                             lambda g=g: emit_ffn(g)]


        if KBISECT == 2:
            bisect_out(hb)
        elif KBISECT == 3:
            bisect_out(hb)
        elif KBISECT == 4:
            bisect_out(x1_rm)


def _host_blobs(inputs):
    Wq = np.asarray(inputs["Wq"], np.float32)
    Wk = np.asarray(inputs["Wk"], np.float32)
    Wv = np.asarray(inputs["Wv"], np.float32)
    Wproj = np.asarray(inputs["Wproj"], np.float32)
    scale = float(HD) ** -0.5

    def pad_heads(W):  # [H, D, HD] -> [32, 128] block layout [d, 32h+hd]
        out = np.zeros((D, P), np.float32)
        for h in range(H):
            out[:, 32 * h : 32 * h + HD] = W[h]
        return out

    wq_pad = np.tile(pad_heads(Wq * scale), (4, 1))
    wk_pad = np.tile(pad_heads(Wk), (4, 1))
    wvp = np.zeros((D, P), np.float32)
    for h in range(H):
        wvp[:, 32 * h : 32 * h + 32] = Wv[h] @ Wproj[HD * h : HD * h + HD]
    wvp = np.tile(wvp, (4, 1))

    bblob = np.zeros((P, NBF), np.float32)
    bblob[:, _WQ : _WQ + P] = wq_pad
    bblob[:, _WK : _WK + P] = wk_pad
    bblob[:, _WVP : _WVP + P] = wvp
    bblob[:, _W1 : _W1 + P] = np.tile(np.asarray(inputs["W1"], np.float32), (4, 1))
    bblob[:, _W2 : _W2 + D] = np.asarray(inputs["W2"], np.float32)
    bblob[:, _IDB : _IDB + P] = np.eye(P, dtype=np.float32)
    bblob[:, _MSK : _MSK + P] = np.triu(np.ones((P, P), np.float32))
    bblob = bblob.astype(ml_dtypes.bfloat16)

    def rep(name):
        return np.tile(np.asarray(inputs[name], np.float32)[None, :], (P, 1))

    fblob = np.zeros((P, NF32), np.float32)
    fblob[:, _G1 : _G1 + D] = rep("ln1_g")
    fblob[:, _B1 : _B1 + D] = rep("ln1_b")
    fblob[:, _G2 : _G2 + D] = rep("ln2_g")
    fblob[:, _LB2 : _LB2 + D] = rep("ln2_b")
    fblob[:, _BPJ : _BPJ + D] = rep("bproj")
    fblob[:, _B2R : _B2R + D] = rep("b2")
    fblob[:, _FB1 : _FB1 + 1] = np.asarray(inputs["b1"], np.float32).reshape(FF, 1)
    fblob[:, _B2C : _B2C + 1] = np.tile(np.asarray(inputs["b2"], np.float32), 4).reshape(P, 1)
    return fblob, bblob


def _get_nc():
    if "nc" not in _NC_CACHE:
        _NC_CACHE["nc"] = _build_nc()
    return _NC_CACHE["nc"]


def _run(inputs, trace=False):
    x = np.asarray(inputs["x"], np.float32)
    fblob, bblob = _host_blobs(inputs)
    nc = _get_nc()
    in_maps = []
    for b in range(B):
        fbm = fblob.copy()
        # x_rm[p, n*32+d] = x[b, 128n+p, d]
        fbm[:, _X0 : _X0 + 512] = (
            x[b].reshape(16, P, D).transpose(1, 0, 2).reshape(P, 512)
        )
        in_maps.append({"fblob": np.ascontiguousarray(fbm), "bblob": bblob})
    res = run_bass_kernel_spmd(nc, in_maps, core_ids=list(range(NCORES)), trace=trace)
    outs = []
    for r in res.results:
        yb = r["y"].astype(np.float32)
        outs.append(yb.reshape(P, 16, D).transpose(1, 0, 2).reshape(T, D))
    return np.stack(outs, axis=0), res


def kernel(**inputs):
    out, _ = _run(inputs)
    return out


def kernel_traced(**inputs):
    out, res = _run(inputs, trace=True)
    return out, res


# revision 60
# speedup vs baseline: 1.0132x; 1.0132x over previous
"""Trainium2 Bass kernel for an 8x(2048,32) decoder block.

Sharding: data-parallel over batch. B=8 batch elements -> 8 NeuronCores,
one batch element per core, parameters replicated, no collectives.

Layouts (t = 512*g + 128*j + p;  n = 4*g + j;  g,j in [0,4), p in [0,128)):
  row-major ("rm"):  tile[p, n, d]
  chunk-transposed ("ct"): tile[32*j + d, (g, p)]
  Q^T/K^T: tile[32*h + hd, (j, g, p)] bf16 (8 valid rows per 32-row block)

Attention (head dim 8, 4 heads):
  S^T[kv, q] per (kv-chunk c, q-group g) via K=8 matmuls, two PSUM tiles of
  2 heads each (double-buffered) so PE can run chunk c+1's S while ScalarE
  still exps chunk c (software-pipelined emission; O matmuls for chunk c
  are emitted after chunk c+1's S matmuls).
  P = exp(S^T): full (off-diagonal) chunks exp to fp8-e4m3 pairs, consumed
  by DoubleRow fp8 matmuls (2 kv-chunks per matmul, 0.5 cyc/col); diagonal
  chunks exp to bf16, triangular mask multiply on VectorE, bf16 matmuls.
  Some full pairs are exp'd on VectorE instead via the Schraudolph int-bias
  trick (x*A+B converted to int8/int16, bit-viewed as fp8/bf16) to offload
  the saturated ScalarE; the ~3% piecewise-linear ripple is harmless here
  (softmax normalization cancels the uniform bias; o' is a small residual
  contribution).
  O'^T[(dout|den), q] += [16*V'_h | 16].T accumulation in PSUM where
  V'_h = Wv[h] @ Wproj rows (folded host-side); the x16 scale puts V' in
  fp8's normal range and cancels in the softmax division.
  Per-g tail: transpose O' back, fused divide/head-sum/residual, LN2 with
  VectorE Newton rsqrt (ScalarE never swaps activation tables), FFN, and
  the output DMA — all pipelined under the next group's attention.
"""

import ml_dtypes
import numpy as np

import concourse.bacc as bacc
import concourse.bass as bass
import concourse.mybir as mybir
import concourse.tile as tile
from concourse.bass import ts
from concourse.bass_utils import run_bass_kernel_spmd

B, T, D, H, HD = 8, 2048, 32, 4, 8
P = 128
NCORES = 8
FF = 4 * D  # 128
FP32 = mybir.dt.float32
BF16 = mybir.dt.bfloat16
FP8 = mybir.dt.float8e4
I8 = mybir.dt.int8
I16 = mybir.dt.int16
I32 = mybir.dt.int32
AF = mybir.ActivationFunctionType
ALU = mybir.AluOpType
AX = mybir.AxisListType
DR = mybir.MatmulPerfMode.DoubleRow
EPS = 1e-5
VSCALE = 16.0

USE_FP8_DR = True
DVE_PAIRS = False
import os
KBISECT = int(os.environ.get("KBISECT", "0"))  # 0=full; 1..4 truncated builds
# Schraudolph exp constants: y = bits(int(x * SCALE + BIAS))
EXP8_SCALE, EXP8_BIAS = 8.0 / np.log(2.0), 56.0 - 0.344
EXP16_SCALE, EXP16_BIAS = 128.0 / np.log(2.0), 16256.0 - 5.5


def _dve_pair(g, pi):
    """Full-chunk pairs routed to VectorE exp (ScalarE offload)."""
    return DVE_PAIRS and (g, pi) in {(2, 3), (3, 4)}


# fp32 blob column offsets (params first so one contiguous DMA covers the
# params + x block g0 that LN1(g0) needs)
_G1 = 0            # ln1_g rep [32]
_B1 = 32           # ln1_b rep [32]
_G2 = 64           # ln2_g rep [32]
_LB2 = 96          # ln2_b rep [32]
_BPJ = 128         # bproj rep [32]
_B2R = 160         # ffn b2 rep [32]
_FB1 = 192         # ffn b1 per-partition [1]
_B2C = 193         # ffn b2 in ct layout (per-partition 32j+d -> b2[d]) [1]
_X0 = 194          # x_rm [512]
NF32 = 706

# bf16 blob column offsets
_WQ = 0            # wq_pad [128]
_WK = 128          # wk_pad [128]
_WVP = 256         # wvp [128]
_W1 = 384          # w1 tiled [128]
_W2 = 512          # w2 [32]
_IDB = 544         # identity [128]
_MSK = 672         # causal mask [128]
NBF = 800

_NC_CACHE = {}


def _build_nc():
    nc = bacc.Bacc(
        "TRN2",
        target_bir_lowering=False,
        debug=False,
        enable_asserts=False,
        num_devices=NCORES,
    )
    bf_d = nc.dram_tensor("bblob", [P, NBF], BF16, kind="ExternalInput").ap()
    f32_d = nc.dram_tensor("fblob", [P, NF32], FP32, kind="ExternalInput").ap()
    y_d = nc.dram_tensor("y", [P, 512], FP32, kind="ExternalOutput").ap()

    with tile.TileContext(nc) as tc:
        _decoder_body(tc, f32_d, bf_d, y_d)
    nc.compile()
    return nc


def _decoder_body(tc, f32_d, bf_d, y_d):
    nc = tc.nc

    with (
        tc.tile_pool(name="pers", bufs=1) as pers,
        tc.tile_pool(name="work", bufs=2) as work,
        tc.tile_pool(name="ps", bufs=1, space="PSUM") as ps,
    ):
        fb = pers.tile([P, NF32], FP32)
        # x block g0 first (LN1's stats only need x), then params, then rest.
        nc.sync.dma_start(fb[:, _X0 : _X0 + 128], f32_d[:, _X0 : _X0 + 128])
        nc.sync.dma_start(fb[:, 0 : _X0], f32_d[:, 0 : _X0])
        bb = pers.tile([P, NBF], BF16)
        nc.sync.dma_start(bb[:], bf_d)
        nc.sync.dma_start(fb[:, _X0 + 128 : NF32], f32_d[:, _X0 + 128 : NF32])

        x3 = fb[:, _X0 : _X0 + 512].rearrange("p (n d) -> p n d", d=D)
        g1 = fb[:, _G1 : _G1 + D]
        b1 = fb[:, _B1 : _B1 + D]
        g2 = fb[:, _G2 : _G2 + D]
        lb2 = fb[:, _LB2 : _LB2 + D]
        bpj = fb[:, _BPJ : _BPJ + D]
        b2r = fb[:, _B2R : _B2R + D]
        fb1 = fb[:, _FB1 : _FB1 + 1]
        b2c = fb[:, _B2C : _B2C + 1]
        wq = bb[:, _WQ : _WQ + P]
        wk = bb[:, _WK : _WK + P]
        wvp = bb[:, _WVP : _WVP + P]
        w1 = bb[:, _W1 : _W1 + P]
        w2 = bb[:, _W2 : _W2 + D]
        idb = bb[:, _IDB : _IDB + P]
        msk = bb[:, _MSK : _MSK + P]

        def rsqrt(out_ap, in_ap, shape, tag, iters=2):
            """out = 1/sqrt(in) via bit-hack + Newton steps (all VectorE)."""
            y0 = work.tile(shape, FP32, tag=tag + "y0", name=tag + "y0")
            sh = work.tile(shape, I32, tag=tag + "sh", name=tag + "sh")
            nc.vector.tensor_scalar(
                sh[:], in_ap.bitcast(I32), 1, None, op0=ALU.logical_shift_right,
            )
            nc.vector.tensor_scalar(
                y0.bitcast(I32), sh[:], -1, 0x5F3759DF, op0=ALU.mult, op1=ALU.add,
            )
            a = work.tile(shape, FP32, tag=tag + "a", name=tag + "a")
            for it in range(iters):
                nc.vector.tensor_mul(a[:], y0[:], y0[:])
                nc.vector.tensor_mul(a[:], a[:], in_ap)
                nc.vector.tensor_scalar(a[:], a[:], -0.5, 1.5, op0=ALU.mult, op1=ALU.add)
                nc.vector.tensor_mul(out_ap if it == iters - 1 else y0[:], y0[:], a[:])

        def layer_norm(src3, g_ap, b_ap, out3, ngrp, tag, ew=None):
            """src3/out3: [P, ngrp, D]; per-(p,group) LN over d. `ew` picks the
            engine for the big elementwise ops (default VectorE)."""
            ew = ew or nc.vector
            mus = work.tile([P, ngrp], FP32, tag=tag + "mu", name=tag + "mu")
            nc.vector.reduce_sum(mus[:], src3, axis=AX.X)
            nc.vector.tensor_scalar(mus[:], mus[:], 1.0 / D, None, op0=ALU.mult)
            xc = work.tile([P, ngrp * D], FP32, tag=tag + "xc", name=tag + "xc")
            xc3 = xc.rearrange("p (n d) -> p n d", d=D)
            ew.tensor_sub(xc3, src3, mus[:, :, None].to_broadcast((P, ngrp, D)))
            sq = work.tile([P, ngrp * D], FP32, tag=tag + "sq", name=tag + "sq")
            sq3 = sq.rearrange("p (n d) -> p n d", d=D)
            ew.tensor_mul(sq3, xc3, xc3)
            vs = work.tile([P, ngrp], FP32, tag=tag + "vs", name=tag + "vs")
            nc.vector.reduce_sum(vs[:], sq3, axis=AX.X)
            nc.vector.tensor_scalar(vs[:], vs[:], 1.0 / D, EPS, op0=ALU.mult, op1=ALU.add)
            istd = work.tile([P, ngrp], FP32, tag=tag + "is", name=tag + "is")
            rsqrt(istd[:], vs[:], [P, ngrp], tag, iters=1 if tag == "lnA" else 2)
            ew.tensor_mul(xc3, xc3, istd[:, :, None].to_broadcast((P, ngrp, D)))
            ew.tensor_mul(xc3, xc3, g_ap[:, None, :].to_broadcast((P, ngrp, D)))
            ew.tensor_add(out3, xc3, b_ap[:, None, :].to_broadcast((P, ngrp, D)))

        h_rm = pers.tile([P, 512], FP32)
        h3 = h_rm.rearrange("p (n d) -> p n d", d=D)
        hb = pers.tile([P, 512], FP32)
        hb3 = hb.rearrange("p (n d) -> p n d", d=D)
        h_bf = pers.tile([P, 512], BF16)
        h_ct = pers.tile([P, 512], BF16)
        qt = pers.tile([P, T], BF16)
        kt = pers.tile([P, T], BF16)
        qt4 = qt.rearrange("p (j g q) -> p j g q", j=4, g=4)
        kt4 = kt.rearrange("p (j g q) -> p j g q", j=4, g=4)
        v8 = pers.tile([P, 16 * H * 64], FP8)
        v84 = v8.rearrange("p (c h e) -> p c h e", c=16, h=H)
        nc.gpsimd.memset(v8.bitcast(I32), 0)
        nc.vector.memset(v84[:, :, :, 32], VSCALE)
        vb4 = None
        if not USE_FP8_DR:
            vb = pers.tile([P, 16 * H * 64], BF16)
            vb4 = vb.rearrange("p (c h e) -> p c h e", c=16, h=H)
            nc.gpsimd.memset(vb.bitcast(I32), 0)
            nc.vector.memset(vb4[:, :, :, 32], VSCALE)

        x1_rm = pers.tile([P, 512], FP32)
        x13 = x1_rm.rearrange("p (n d) -> p n d", d=D)
        h2_rm = pers.tile([P, 512], FP32)
        h23 = h2_rm.rearrange("p (n d) -> p n d", d=D)
        h2_bf = pers.tile([P, 512], BF16)
        h2b3 = h2_bf.rearrange("p (n d) -> p n d", d=D)
        a_sb = pers.tile([FF, 16 * P], BF16)
        a_sb3 = a_sb.rearrange("f (n p) -> f n p", n=16)
        y_rm = pers.tile([P, 512], FP32)
        y3 = y_rm.rearrange("p (n d) -> p n d", d=D)

        def emit_ln1(n0, n1, tag, ew=None):
            layer_norm(x3[:, n0:n1, :], g1, b1, h3[:, n0:n1, :], n1 - n0, tag, ew=ew)
            nc.vector.tensor_add(
                hb3[:, n0:n1, :], h3[:, n0:n1, :],
                bpj[:, None, :].to_broadcast((P, n1 - n0, D)))

        def emit_hct(gs):
            nc.scalar.activation(
                h_bf[:, 128 * gs[0] : 128 * (gs[-1] + 1)],
                h_rm[:, 128 * gs[0] : 128 * (gs[-1] + 1)], AF.Copy)
            hct_ps = ps.tile([P, len(gs) * P], BF16, tag="t", name="hct_ps")
            for i, g in enumerate(gs):
                nc.tensor.transpose(hct_ps[:, ts(i, P)], h_bf[:, ts(g, P)], idb)
            nc.scalar.activation(
                h_ct[:, 128 * gs[0] : 128 * (gs[-1] + 1)], hct_ps[:], AF.Copy)

        def qkv_pieces(g):
            """Per-j emission pieces for Q^T/K^T cols (:, g, :) and V'
            chunks c=4g+j. Each matmul gets its own PSUM tile -- matmuls
            with different tile_position into one PSUM bank fault on HW."""
            def mk_q(j):
                def f():
                    q_ps = ps.tile([P, P], FP32, tag="t", name="q_ps")
                    nc.tensor.matmul(
                        q_ps[:], lhsT=wq[ts(j, 32), :],
                        rhs=h_ct[ts(j, 32), ts(g, P)],
                        start=True, stop=True, tile_position=(32 * j, 0))
                    if g <= 1:
                        nc.scalar.activation(qt4[:, j, g, :], q_ps[:], AF.Copy)
                    else:
                        nc.vector.tensor_copy(qt4[:, j, g, :], q_ps[:])
                return f

            def mk_k(j):
                def f():
                    k_ps = ps.tile([P, P], FP32, tag="t", name="k_ps")
                    nc.tensor.matmul(
                        k_ps[:], lhsT=wk[ts(j, 32), :],
                        rhs=h_ct[ts(j, 32), ts(g, P)],
                        start=True, stop=True, tile_position=(32 * j, 0))
                    if g <= 1:
                        nc.scalar.activation(kt4[:, j, g, :], k_ps[:], AF.Copy)
                    else:
                        nc.vector.tensor_copy(kt4[:, j, g, :], k_ps[:])
                return f

            def mk_v(j):
                def f():
                    c = 4 * g + j
                    vp_ps = ps.tile([P, P], FP32, tag="t", name="vp_ps")
                    nc.tensor.matmul(
                        vp_ps[:], lhsT=h_ct[ts(j, 32), ts(g, P)],
                        rhs=wvp[ts(j, 32), :],
                        start=True, stop=True, tile_position=(32 * j, 0))
                    vps = vp_ps.rearrange("p (h e) -> p h e", e=32)
                    if g <= 1:
                        nc.scalar.activation(v84[:, c, :, 0:32], vps, AF.Copy,
                                             scale=VSCALE)
                    else:
                        nc.vector.tensor_scalar(
                            v84[:, c, :, 0:32], vps, VSCALE, None, op0=ALU.mult)
                return f

            out = [mk_q(j) for j in range(4)] + [mk_k(j) for j in range(4)]
            if KBISECT != 15:
                out += [mk_v(j) for j in range(4)]
            return out

        def emit_qkv(g):
            for f in qkv_pieces(g):
                f()
                if not USE_FP8_DR:
                    nc.vector.tensor_scalar(
                        vb4[:, c, :, 0:32], vps, VSCALE, None, op0=ALU.mult)

        def emit_ffn(gg):
            """h2 block gg -> y block gg -> DMA out."""
            h2t = ps.tile([P, P], BF16, tag="t", name="h2t")
            nc.tensor.transpose(h2t[:], h2_bf[:, ts(gg, P)], idb)
            h2c = work.tile([P, P], BF16, tag="h2c", name="h2c")
            nc.vector.tensor_copy(h2c[:], h2t[:])
            for j in range(4):
                a_ps = ps.tile([P, P], FP32, tag="t", name="a_ps")
                nc.tensor.matmul(
                    a_ps[:], lhsT=w1[ts(j, 32), :], rhs=h2c[ts(j, 32), :],
                    start=True, stop=True, tile_position=(32 * j, 0))
                nc.vector.tensor_scalar(
                    a_sb3[:, 4 * gg + j, :], a_ps[:],
                    fb1, 0.0, op0=ALU.add, op1=ALU.max)
            ff_ps = ps.tile([P, P], FP32, tag="t", name="ff_ps")
            for j in range(4):
                nc.tensor.matmul(
                    ff_ps[ts(j, 32), :], lhsT=w2, rhs=a_sb3[:, 4 * gg + j, :],
                    start=True, stop=True, tile_position=(0, 32 * j))
            ffb = work.tile([P, P], BF16, tag="ffb", name="ffb")
            nc.vector.tensor_scalar(ffb[:], ff_ps[:], b2c, None, op0=ALU.add)
            ftp = ps.tile([P, P], BF16, tag="t", name="ftp")
            nc.tensor.transpose(ftp[:], ffb[:], idb)
            yb = y3[:, 4 * gg : 4 * gg + 4, :]
            nc.vector.tensor_add(yb, ftp.rearrange("p (n d) -> p n d", d=D),
                                 h23[:, 4 * gg : 4 * gg + 4, :])
            nc.sync.dma_start(y_d[:, ts(gg, P)], y_rm[:, ts(gg, P)])

        def bisect_out(src_tile):
            nc.vector.tensor_copy(y_rm[:], src_tile[:])
            nc.sync.dma_start(y_d, y_rm[:])

        # ---- attention + per-g postprocessing, software-pipelined ----
        pend = []
        pend_tail = []
        for g in range(4):
            if g == 1:
                emit_hct([1, 2, 3])
                emit_qkv(1)
                emit_qkv(2)
            elif g == 2:
                emit_qkv(3)
            if g == 0:
                if KBISECT == 11:
                    nc.vector.tensor_copy(y_rm[:], fb[:, _X0 : _X0 + 512])
                    nc.sync.dma_start(y_d, y_rm[:])
                    return
                emit_ln1(0, 4, "lnA")
                if KBISECT == 12:
                    bisect_out(h_rm)
                    return
                emit_hct([0])
                if KBISECT == 13:
                    bisect_out(h_rm)
                    return
                emit_qkv(0)
                if KBISECT in (14, 15, 16):
                    bisect_out(h_rm)
                    return
                if KBISECT == 1:
                    emit_ln1(4, 16, "lnC")
                    bisect_out(h_rm)
                    return

            oA = ps.tile([P, 512], FP32, tag="o", bufs=2, name="oA")
            oB = ps.tile([P, 512], FP32, tag="o", bufs=2, name="oB")
            nch = 4 * g + 4
            p8_live = None
            for c in range(nch):
                m = c - 4 * g
                diag = m >= 0
                lo = 128 * m if diag else 0
                loj = lo // 128
                gc_, jc = divmod(c, 4)
                s0 = ps.tile([P, 2 * 512], FP32, tag="s", bufs=2, name="s0")
                s03 = s0.rearrange("p (h q) -> p h q", h=2)
                s1 = ps.tile([P, 2 * 512], FP32, tag="s", bufs=2, name="s1")
                s13 = s1.rearrange("p (h q) -> p h q", h=2)
                for h in range(4):
                    st = s03 if h < 2 else s13
                    nc.tensor.matmul(
                        st[:, h % 2, lo:],
                        lhsT=kt4[32 * h : 32 * h + HD, jc, gc_, :],
                        rhs=qt4[32 * h : 32 * h + HD, loj:, g, :],
                        start=True, stop=True,
                        tile_position=(32 * h, 0),
                    )
                for fn in pend:
                    fn()
                pend = []
                if diag and m == 2 and KBISECT != 3:
                    mk_half(g, oA, oB, 0)()
                    if g == 3:
                        mk_ln2h(g, 0)()
                if pend_tail:
                    pend_tail.pop(0)()
                if g == 0 and c == 1:
                    # LN1 for groups 1..3: scheduled behind g0's critical
                    # chain (the tile scheduler would otherwise interleave it
                    # into LN1-g0 on VectorE and delay the whole pipeline),
                    # with the big elementwise ops on the idle GpSimd engine.
                    with tc.tile_wait_until(0.009):
                        emit_ln1(4, 16, "lnB", ew=nc.gpsimd)
                if diag:
                    pd = work.tile([P, 4 * 512], BF16, tag="pd", bufs=4, name="pd")
                    pd3 = pd.rearrange("p (h q) -> p h q", h=4)
                    nc.scalar.activation(pd3[:, 0:2, lo:], s03[:, :, lo:], AF.Exp)
                    nc.scalar.activation(pd3[:, 2:4, lo:], s13[:, :, lo:], AF.Exp)
                    nc.vector.tensor_mul(
                        pd3[:, :, lo : lo + P],
                        pd3[:, :, lo : lo + P],
                        msk[:, None, :].to_broadcast((P, 4, P)),
                    )

                    def mk_diag(c=c, lo=lo, pd3=pd3, oA=oA, oB=oB, nch=nch):
                        def f():
                            vsrc = v84 if USE_FP8_DR else vb4
                            for h in range(4):
                                ob = oA if h < 2 else oB
                                base = 64 * (h % 2)
                                nc.tensor.matmul(
                                    ob[base : base + 64, lo:],
                                    lhsT=vsrc[:, c, h, :],
                                    rhs=pd3[:, h, lo:],
                                    start=(c == 0),
                                    stop=(c == nch - 1),
                                    skip_group_check=True,
                                )
                        return f

                    pend.append(mk_diag())
                elif USE_FP8_DR:
                    par = c & 1
                    if par == 0:
                        p8_live = work.tile([P, 2 * 4 * 512], FP8, tag="p8",
                                            bufs=3, name="p8")
                    p84 = p8_live.rearrange("p (r h q) -> p r h q", r=2, h=4)
                    if _dve_pair(g, c // 2):
                        nc.vector.tensor_scalar(
                            p84.bitcast(I8)[:, par, 0:2, :], s03,
                            EXP8_SCALE, EXP8_BIAS, op0=ALU.mult, op1=ALU.add)
                        nc.vector.tensor_scalar(
                            p84.bitcast(I8)[:, par, 2:4, :], s13,
                            EXP8_SCALE, EXP8_BIAS, op0=ALU.mult, op1=ALU.add)
                    else:
                        nc.scalar.activation(p84[:, par, 0:2, :], s03, AF.Exp)
                        nc.scalar.activation(p84[:, par, 2:4, :], s13, AF.Exp)
                    if par == 1:
                        def mk_pair(c=c, p84=p84, oA=oA, oB=oB):
                            def f():
                                # even heads: DoubleRow fp8 (tile position 0
                                # only -- walrus rejects DR at col offset 64);
                                # odd heads: plain fp8 matmuls per chunk.
                                for h in (0, 2):
                                    ob = oA if h < 2 else oB
                                    for qh in range(2):
                                        nc.tensor.matmul(
                                            ob[0:64, ts(qh, 256)],
                                            lhsT=v84[:, c - 1 : c + 1, h, :],
                                            rhs=p84[:, :, h, ts(qh, 256)],
                                            perf_mode=DR,
                                            start=(c == 1),
                                            stop=False,
                                            skip_group_check=True,
                                        )
                                for h in (1, 3):
                                    ob = oA if h < 2 else oB
                                    for cc in (c - 1, c):
                                        nc.tensor.matmul(
                                            ob[64:128, :],
                                            lhsT=v84[:, cc, h, :],
                                            rhs=p84[:, cc & 1, h, :],
                                            start=(cc == 0),
                                            stop=False,
                                            tile_position=(0, 64),
                                            skip_group_check=True,
                                        )
                            return f

                        pend.append(mk_pair())
                else:
                    pdf = work.tile([P, 4 * 512], BF16, tag="pd", bufs=4, name="pdf")
                    pdf3 = pdf.rearrange("p (h q) -> p h q", h=4)
                    nc.scalar.activation(pdf3[:, 0:2, :], s03, AF.Exp)
                    nc.scalar.activation(pdf3[:, 2:4, :], s13, AF.Exp)

                    def mk_full(c=c, pdf3=pdf3, oA=oA, oB=oB):
                        def f():
                            for h in range(4):
                                ob = oA if h < 2 else oB
                                base = 64 * (h % 2)
                                nc.tensor.matmul(
                                    ob[base : base + 64, :],
                                    lhsT=vb4[:, c, h, :],
                                    rhs=pdf3[:, h, :],
                                    start=(c == 0),
                                    stop=False,
                                    skip_group_check=True,
                                )
                        return f

                    pend.append(mk_full())
            for fn in pend:
                fn()
            pend = []

            def mk_half(g, oA, oB, half):
                def f():
                    # finalize q'-columns [256*half, 256*half+256) of O':
                    # transpose back, divide by den, sum heads, residual.
                    osbA = work.tile([P, 256], BF16, tag="osb", bufs=4,
                                     name="osbA")
                    nc.vector.tensor_copy(osbA[:], oA[:, ts(half, 256)])
                    osbB = work.tile([P, 256], BF16, tag="osb", bufs=4,
                                     name="osbB")
                    nc.vector.tensor_copy(osbB[:], oB[:, ts(half, 256)])
                    otpA = ps.tile([P, 256], BF16, tag="t", name="otpA")
                    otpB = ps.tile([P, 256], BF16, tag="t", name="otpB")
                    for jj in range(2):
                        nc.tensor.transpose(otpA[:, ts(jj, P)],
                                            osbA[:, ts(jj, P)], idb)
                    for jj in range(2):
                        nc.tensor.transpose(otpB[:, ts(jj, P)],
                                            osbB[:, ts(jj, P)], idb)
                    oa4 = otpA.rearrange("p (j k e) -> p j k e", j=2, k=2)
                    ob4 = otpB.rearrange("p (j k e) -> p j k e", j=2, k=2)
                    drA = work.tile([P, 2 * 2], FP32, tag="dr", name="drA")
                    drA3 = drA.rearrange("p (j k) -> p j k", j=2)
                    nc.vector.reciprocal(drA3, oa4[:, :, :, 32])
                    drB = work.tile([P, 2 * 2], FP32, tag="dr", name="drB")
                    drB3 = drB.rearrange("p (j k) -> p j k", j=2)
                    nc.vector.reciprocal(drB3, ob4[:, :, :, 32])
                    tmA = work.tile([P, 2 * 2 * 32], FP32, tag="tm", name="tmA")
                    tmA4 = tmA.rearrange("p (j k e) -> p j k e", j=2, k=2)
                    nc.vector.tensor_mul(
                        tmA4, oa4[:, :, :, 0:32],
                        drA3[:, :, :, None].to_broadcast((P, 2, 2, 32)))
                    tmB = work.tile([P, 2 * 2 * 32], FP32, tag="tm", name="tmB")
                    tmB4 = tmB.rearrange("p (j k e) -> p j k e", j=2, k=2)
                    nc.vector.tensor_mul(
                        tmB4, ob4[:, :, :, 0:32],
                        drB3[:, :, :, None].to_broadcast((P, 2, 2, 32)))
                    n0 = 4 * g + 2 * half
                    u1 = work.tile([P, 2 * 32], FP32, tag="u", name="u1")
                    u13 = u1.rearrange("p (j e) -> p j e", j=2)
                    nc.vector.tensor_add(u13, tmA4[:, :, 0, :], tmA4[:, :, 1, :])
                    u2 = work.tile([P, 2 * 32], FP32, tag="u", name="u2")
                    u23 = u2.rearrange("p (j e) -> p j e", j=2)
                    nc.vector.tensor_add(u23, tmB4[:, :, 0, :], tmB4[:, :, 1, :])
                    nc.vector.tensor_add(u13, u13, u23)
                    nc.vector.tensor_add(x13[:, n0 : n0 + 2, :], u13,
                                         hb3[:, n0 : n0 + 2, :])
                return f

            def mk_ln2(g):
                def f():
                    layer_norm(x13[:, 4 * g : 4 * g + 4, :], g2, lb2,
                               h23[:, 4 * g : 4 * g + 4, :], 4, "ln2")
                    nc.vector.tensor_copy(h2b3[:, 4 * g : 4 * g + 4, :],
                                          h23[:, 4 * g : 4 * g + 4, :])
                return f

            def mk_ln2h(g, half):
                def f():
                    n0 = 4 * g + 2 * half
                    layer_norm(x13[:, n0 : n0 + 2, :], g2, lb2,
                               h23[:, n0 : n0 + 2, :], 2, "ln2")
                    nc.vector.tensor_copy(h2b3[:, n0 : n0 + 2, :],
                                          h23[:, n0 : n0 + 2, :])
                return f

            if KBISECT == 3:
                pend_tail = []
            elif KBISECT == 4:
                pend_tail = [mk_half(g, oA, oB, 1)]
                if g == 3:
                    pend_tail[0]()
                    pend_tail = []
            elif g == 3:
                mk_half(g, oA, oB, 1)()
                mk_ln2h(g, 1)()
                emit_ffn(g)
                pend_tail = []
            else:
                pend_tail = [mk_half(g, oA, oB, 1), mk_ln2(g),
                             lambda g=g: emit_ffn(g)]


        if KBISECT == 2:
            bisect_out(hb)
        elif KBISECT == 3:
            bisect_out(hb)
        elif KBISECT == 4:
            bisect_out(x1_rm)


def _host_blobs(inputs):
    Wq = np.asarray(inputs["Wq"], np.float32)
    Wk = np.asarray(inputs["Wk"], np.float32)
    Wv = np.asarray(inputs["Wv"], np.float32)
    Wproj = np.asarray(inputs["Wproj"], np.float32)
    scale = float(HD) ** -0.5

    def pad_heads(W):  # [H, D, HD] -> [32, 128] block layout [d, 32h+hd]
        out = np.zeros((D, P), np.float32)
        for h in range(H):
            out[:, 32 * h : 32 * h + HD] = W[h]
        return out

    wq_pad = np.tile(pad_heads(Wq * scale), (4, 1))
    wk_pad = np.tile(pad_heads(Wk), (4, 1))
    wvp = np.zeros((D, P), np.float32)
    for h in range(H):
        wvp[:, 32 * h : 32 * h + 32] = Wv[h] @ Wproj[HD * h : HD * h + HD]
    wvp = np.tile(wvp, (4, 1))

    bblob = np.zeros((P, NBF), np.float32)
    bblob[:, _WQ : _WQ + P] = wq_pad
    bblob[:, _WK : _WK + P] = wk_pad
    bblob[:, _WVP : _WVP + P] = wvp
    bblob[:, _W1 : _W1 + P] = np.tile(np.asarray(inputs["W1"], np.float32), (4, 1))
    bblob[:, _W2 : _W2 + D] = np.asarray(inputs["W2"], np.float32)
    bblob[:, _IDB : _IDB + P] = np.eye(P, dtype=np.float32)
    bblob[:, _MSK : _MSK + P] = np.triu(np.ones((P, P), np.float32))
    bblob = bblob.astype(ml_dtypes.bfloat16)

    def rep(name):
        return np.tile(np.asarray(inputs[name], np.float32)[None, :], (P, 1))

    fblob = np.zeros((P, NF32), np.float32)
    fblob[:, _G1 : _G1 + D] = rep("ln1_g")
    fblob[:, _B1 : _B1 + D] = rep("ln1_b")
    fblob[:, _G2 : _G2 + D] = rep("ln2_g")
    fblob[:, _LB2 : _LB2 + D] = rep("ln2_b")
    fblob[:, _BPJ : _BPJ + D] = rep("bproj")
    fblob[:, _B2R : _B2R + D] = rep("b2")
    fblob[:, _FB1 : _FB1 + 1] = np.asarray(inputs["b1"], np.float32).reshape(FF, 1)
    fblob[:, _B2C : _B2C + 1] = np.tile(np.asarray(inputs["b2"], np.float32), 4).reshape(P, 1)
    return fblob, bblob


def _get_nc():
    if "nc" not in _NC_CACHE:
        _NC_CACHE["nc"] = _build_nc()
    return _NC_CACHE["nc"]


def _run(inputs, trace=False):
    x = np.asarray(inputs["x"], np.float32)
    fblob, bblob = _host_blobs(inputs)
    nc = _get_nc()
    in_maps = []
    for b in range(B):
        fbm = fblob.copy()
        # x_rm[p, n*32+d] = x[b, 128n+p, d]
        fbm[:, _X0 : _X0 + 512] = (
            x[b].reshape(16, P, D).transpose(1, 0, 2).reshape(P, 512)
        )
        in_maps.append({"fblob": np.ascontiguousarray(fbm), "bblob": bblob})
    res = run_bass_kernel_spmd(nc, in_maps, core_ids=list(range(NCORES)), trace=trace)
    outs = []
    for r in res.results:
        yb = r["y"].astype(np.float32)
        outs.append(yb.reshape(P, 16, D).transpose(1, 0, 2).reshape(T, D))
    return np.stack(outs, axis=0), res


def kernel(**inputs):
    out, _ = _run(inputs)
    return out


def kernel_traced(**inputs):
    out, res = _run(inputs, trace=True)
    return out, res


# revision 65
# speedup vs baseline: 1.0342x; 1.0207x over previous
"""Trainium2 Bass kernel for an 8x(2048,32) decoder block.

Sharding: data-parallel over batch. B=8 batch elements -> 8 NeuronCores,
one batch element per core, parameters replicated, no collectives.

Layouts (t = 512*g + 128*j + p;  n = 4*g + j;  g,j in [0,4), p in [0,128)):
  row-major ("rm"):  tile[p, n, d]
  chunk-transposed ("ct"): tile[32*j + d, (g, p)]
  Q^T/K^T: tile[32*h + hd, (j, g, p)] bf16 (8 valid rows per 32-row block)

Attention (head dim 8, 4 heads):
  S^T[kv, q] per (kv-chunk c, q-group g) via K=8 matmuls, two PSUM tiles of
  2 heads each (double-buffered) so PE can run chunk c+1's S while ScalarE
  still exps chunk c (software-pipelined emission; O matmuls for chunk c
  are emitted after chunk c+1's S matmuls).
  P = exp(S^T): full (off-diagonal) chunks exp to fp8-e4m3 pairs, consumed
  by DoubleRow fp8 matmuls (2 kv-chunks per matmul, 0.5 cyc/col); diagonal
  chunks exp to bf16, triangular mask multiply on VectorE, bf16 matmuls.
  Some full pairs are exp'd on VectorE instead via the Schraudolph int-bias
  trick (x*A+B converted to int8/int16, bit-viewed as fp8/bf16) to offload
  the saturated ScalarE; the ~3% piecewise-linear ripple is harmless here
  (softmax normalization cancels the uniform bias; o' is a small residual
  contribution).
  O'^T[(dout|den), q] += [16*V'_h | 16].T accumulation in PSUM where
  V'_h = Wv[h] @ Wproj rows (folded host-side); the x16 scale puts V' in
  fp8's normal range and cancels in the softmax division.
  Per-g tail: transpose O' back, fused divide/head-sum/residual, LN2 with
  VectorE Newton rsqrt (ScalarE never swaps activation tables), FFN, and
  the output DMA — all pipelined under the next group's attention.
"""

import ml_dtypes
import numpy as np

import concourse.bacc as bacc
import concourse.bass as bass
import concourse.mybir as mybir
import concourse.tile as tile
from concourse.bass import ts
from concourse.bass_utils import run_bass_kernel_spmd

B, T, D, H, HD = 8, 2048, 32, 4, 8
P = 128
NCORES = 8
FF = 4 * D  # 128
FP32 = mybir.dt.float32
BF16 = mybir.dt.bfloat16
FP8 = mybir.dt.float8e4
I8 = mybir.dt.int8
I16 = mybir.dt.int16
I32 = mybir.dt.int32
AF = mybir.ActivationFunctionType
ALU = mybir.AluOpType
AX = mybir.AxisListType
DR = mybir.MatmulPerfMode.DoubleRow
EPS = 1e-5
VSCALE = 16.0

USE_FP8_DR = True
DVE_PAIRS = False
import os
KBISECT = int(os.environ.get("KBISECT", "0"))  # 0=full; 1..4 truncated builds
# Schraudolph exp constants: y = bits(int(x * SCALE + BIAS))
EXP8_SCALE, EXP8_BIAS = 8.0 / np.log(2.0), 56.0 - 0.344
EXP16_SCALE, EXP16_BIAS = 128.0 / np.log(2.0), 16256.0 - 5.5


def _dve_pair(g, pi):
    """Full-chunk pairs routed to VectorE exp (ScalarE offload)."""
    return DVE_PAIRS and (g, pi) in {(2, 3), (3, 4)}


# fp32 blob column offsets (params first so one contiguous DMA covers the
# params + x block g0 that LN1(g0) needs)
_G1 = 0            # ln1_g rep [32]
_B1 = 32           # ln1_b rep [32]
_G2 = 64           # ln2_g rep [32]
_LB2 = 96          # ln2_b rep [32]
_BPJ = 128         # bproj rep [32]
_B2R = 160         # ffn b2 rep [32]
_FB1 = 192         # ffn b1 per-partition [1]
_B2C = 193         # ffn b2 in ct layout (per-partition 32j+d -> b2[d]) [1]
_X0 = 194          # x_rm [512]
NF32 = 706

# bf16 blob column offsets
_WQ = 0            # wq_pad [128]
_WK = 128          # wk_pad [128]
_WVP = 256         # wvp [128]
_W1 = 384          # w1 tiled [128]
_W2 = 512          # w2 [32]
_IDB = 544         # identity [128]
_MSK = 672         # causal mask [128]
NBF = 800

_NC_CACHE = {}


def _build_nc():
    nc = bacc.Bacc(
        "TRN2",
        target_bir_lowering=False,
        debug=False,
        enable_asserts=False,
        num_devices=NCORES,
    )
    bf_d = nc.dram_tensor("bblob", [P, NBF], BF16, kind="ExternalInput").ap()
    f32_d = nc.dram_tensor("fblob", [P, NF32], FP32, kind="ExternalInput").ap()
    y_d = nc.dram_tensor("y", [P, 512], FP32, kind="ExternalOutput").ap()

    with tile.TileContext(nc) as tc:
        _decoder_body(tc, f32_d, bf_d, y_d)
    nc.compile()
    return nc


def _decoder_body(tc, f32_d, bf_d, y_d):
    nc = tc.nc

    with (
        tc.tile_pool(name="pers", bufs=1) as pers,
        tc.tile_pool(name="work", bufs=2) as work,
        tc.tile_pool(name="ps", bufs=1, space="PSUM") as ps,
    ):
        fb = pers.tile([P, NF32], FP32)
        # x block g0 first (LN1's stats only need x), then params, then rest.
        nc.sync.dma_start(fb[:, _X0 : _X0 + 128], f32_d[:, _X0 : _X0 + 128])
        nc.sync.dma_start(fb[:, 0 : _X0], f32_d[:, 0 : _X0])
        bb = pers.tile([P, NBF], BF16)
        nc.sync.dma_start(bb[:], bf_d)
        nc.sync.dma_start(fb[:, _X0 + 128 : NF32], f32_d[:, _X0 + 128 : NF32])

        x3 = fb[:, _X0 : _X0 + 512].rearrange("p (n d) -> p n d", d=D)
        g1 = fb[:, _G1 : _G1 + D]
        b1 = fb[:, _B1 : _B1 + D]
        g2 = fb[:, _G2 : _G2 + D]
        lb2 = fb[:, _LB2 : _LB2 + D]
        bpj = fb[:, _BPJ : _BPJ + D]
        b2r = fb[:, _B2R : _B2R + D]
        fb1 = fb[:, _FB1 : _FB1 + 1]
        b2c = fb[:, _B2C : _B2C + 1]
        wq = bb[:, _WQ : _WQ + P]
        wk = bb[:, _WK : _WK + P]
        wvp = bb[:, _WVP : _WVP + P]
        w1 = bb[:, _W1 : _W1 + P]
        w2 = bb[:, _W2 : _W2 + D]
        idb = bb[:, _IDB : _IDB + P]
        msk = bb[:, _MSK : _MSK + P]

        def rsqrt(out_ap, in_ap, shape, tag, iters=2):
            """out = 1/sqrt(in) via bit-hack + Newton steps (all VectorE)."""
            y0 = work.tile(shape, FP32, tag=tag + "y0", name=tag + "y0")
            sh = work.tile(shape, I32, tag=tag + "sh", name=tag + "sh")
            nc.vector.tensor_scalar(
                sh[:], in_ap.bitcast(I32), 1, None, op0=ALU.logical_shift_right,
            )
            nc.vector.tensor_scalar(
                y0.bitcast(I32), sh[:], -1, 0x5F3759DF, op0=ALU.mult, op1=ALU.add,
            )
            a = work.tile(shape, FP32, tag=tag + "a", name=tag + "a")
            for it in range(iters):
                nc.vector.tensor_mul(a[:], y0[:], y0[:])
                nc.vector.tensor_mul(a[:], a[:], in_ap)
                nc.vector.tensor_scalar(a[:], a[:], -0.5, 1.5, op0=ALU.mult, op1=ALU.add)
                nc.vector.tensor_mul(out_ap if it == iters - 1 else y0[:], y0[:], a[:])

        def layer_norm(src3, g_ap, b_ap, out3, ngrp, tag, ew=None):
            """src3/out3: [P, ngrp, D]; per-(p,group) LN over d. `ew` picks the
            engine for the big elementwise ops (default VectorE)."""
            ew = ew or nc.vector
            mus = work.tile([P, ngrp], FP32, tag=tag + "mu", name=tag + "mu")
            nc.vector.reduce_sum(mus[:], src3, axis=AX.X)
            nc.vector.tensor_scalar(mus[:], mus[:], 1.0 / D, None, op0=ALU.mult)
            xc = work.tile([P, ngrp * D], FP32, tag=tag + "xc", name=tag + "xc")
            xc3 = xc.rearrange("p (n d) -> p n d", d=D)
            ew.tensor_sub(xc3, src3, mus[:, :, None].to_broadcast((P, ngrp, D)))
            sq = work.tile([P, ngrp * D], FP32, tag=tag + "sq", name=tag + "sq")
            sq3 = sq.rearrange("p (n d) -> p n d", d=D)
            ew.tensor_mul(sq3, xc3, xc3)
            vs = work.tile([P, ngrp], FP32, tag=tag + "vs", name=tag + "vs")
            nc.vector.reduce_sum(vs[:], sq3, axis=AX.X)
            nc.vector.tensor_scalar(vs[:], vs[:], 1.0 / D, EPS, op0=ALU.mult, op1=ALU.add)
            istd = work.tile([P, ngrp], FP32, tag=tag + "is", name=tag + "is")
            rsqrt(istd[:], vs[:], [P, ngrp], tag, iters=1 if tag == "lnA" else 2)
            ew.tensor_mul(xc3, xc3, istd[:, :, None].to_broadcast((P, ngrp, D)))
            ew.tensor_mul(xc3, xc3, g_ap[:, None, :].to_broadcast((P, ngrp, D)))
            ew.tensor_add(out3, xc3, b_ap[:, None, :].to_broadcast((P, ngrp, D)))

        h_rm = pers.tile([P, 512], FP32)
        h3 = h_rm.rearrange("p (n d) -> p n d", d=D)
        hb = pers.tile([P, 512], FP32)
        hb3 = hb.rearrange("p (n d) -> p n d", d=D)
        h_bf = pers.tile([P, 512], BF16)
        h_ct = pers.tile([P, 512], BF16)
        qt = pers.tile([P, T], BF16)
        kt = pers.tile([P, T], BF16)
        qt4 = qt.rearrange("p (j g q) -> p j g q", j=4, g=4)
        kt4 = kt.rearrange("p (j g q) -> p j g q", j=4, g=4)
        v8 = pers.tile([P, 16 * H * 64], FP8)
        v84 = v8.rearrange("p (c h e) -> p c h e", c=16, h=H)
        nc.gpsimd.memset(v8.bitcast(I32), 0)
        nc.vector.memset(v84[:, :, :, 32], VSCALE)
        vb4 = None
        if not USE_FP8_DR:
            vb = pers.tile([P, 16 * H * 64], BF16)
            vb4 = vb.rearrange("p (c h e) -> p c h e", c=16, h=H)
            nc.gpsimd.memset(vb.bitcast(I32), 0)
            nc.vector.memset(vb4[:, :, :, 32], VSCALE)

        x1_rm = pers.tile([P, 512], FP32)
        x13 = x1_rm.rearrange("p (n d) -> p n d", d=D)
        h2_rm = pers.tile([P, 512], FP32)
        h23 = h2_rm.rearrange("p (n d) -> p n d", d=D)
        h2_bf = pers.tile([P, 512], BF16)
        h2b3 = h2_bf.rearrange("p (n d) -> p n d", d=D)
        a_sb = pers.tile([FF, 16 * P], BF16)
        a_sb3 = a_sb.rearrange("f (n p) -> f n p", n=16)
        y_rm = pers.tile([P, 512], FP32)
        y3 = y_rm.rearrange("p (n d) -> p n d", d=D)

        def emit_ln1(n0, n1, tag, ew=None):
            layer_norm(x3[:, n0:n1, :], g1, b1, h3[:, n0:n1, :], n1 - n0, tag, ew=ew)
            nc.vector.tensor_add(
                hb3[:, n0:n1, :], h3[:, n0:n1, :],
                bpj[:, None, :].to_broadcast((P, n1 - n0, D)))

        def emit_hct(gs):
            nc.scalar.activation(
                h_bf[:, 128 * gs[0] : 128 * (gs[-1] + 1)],
                h_rm[:, 128 * gs[0] : 128 * (gs[-1] + 1)], AF.Copy)
            hct_ps = ps.tile([P, len(gs) * P], BF16, tag="t", name="hct_ps")
            for i, g in enumerate(gs):
                nc.tensor.transpose(hct_ps[:, ts(i, P)], h_bf[:, ts(g, P)], idb)
            nc.scalar.activation(
                h_ct[:, 128 * gs[0] : 128 * (gs[-1] + 1)], hct_ps[:], AF.Copy)

        def qkv_pieces(g):
            """Per-j emission pieces for Q^T/K^T cols (:, g, :) and V'
            chunks c=4g+j. Each matmul gets its own PSUM tile -- matmuls
            with different tile_position into one PSUM bank fault on HW."""
            def mk_q(j):
                def f():
                    q_ps = ps.tile([P, P], FP32, tag="t", name="q_ps")
                    nc.tensor.matmul(
                        q_ps[:], lhsT=wq[ts(j, 32), :],
                        rhs=h_ct[ts(j, 32), ts(g, P)],
                        start=True, stop=True, tile_position=(32 * j, 0))
                    if g <= 1:
                        nc.scalar.activation(qt4[:, j, g, :], q_ps[:], AF.Copy)
                    else:
                        nc.vector.tensor_copy(qt4[:, j, g, :], q_ps[:])
                return f

            def mk_k(j):
                def f():
                    # tag "o" ring: idle during the qkv window, so the K chain
                    # pipelines independently of the Q chain's "t" ring
                    k_ps = ps.tile([P, P], FP32, tag="o", bufs=2, name="k_ps")
                    nc.tensor.matmul(
                        k_ps[:], lhsT=wk[ts(j, 32), :],
                        rhs=h_ct[ts(j, 32), ts(g, P)],
                        start=True, stop=True, tile_position=(32 * j, 0))
                    if g <= 1:
                        nc.scalar.activation(kt4[:, j, g, :], k_ps[:], AF.Copy)
                    else:
                        nc.vector.tensor_copy(kt4[:, j, g, :], k_ps[:])
                return f

            def mk_v(j):
                def f():
                    c = 4 * g + j
                    vp_ps = ps.tile([P, P], FP32, tag="t", name="vp_ps")
                    nc.tensor.matmul(
                        vp_ps[:], lhsT=h_ct[ts(j, 32), ts(g, P)],
                        rhs=wvp[ts(j, 32), :],
                        start=True, stop=True, tile_position=(32 * j, 0))
                    vps = vp_ps.rearrange("p (h e) -> p h e", e=32)
                    if g <= 1:
                        nc.scalar.activation(v84[:, c, :, 0:32], vps, AF.Copy,
                                             scale=VSCALE)
                    else:
                        nc.vector.tensor_scalar(
                            v84[:, c, :, 0:32], vps, VSCALE, None, op0=ALU.mult)
                return f

            out = [mk_q(j) for j in range(4)] + [mk_k(j) for j in range(4)]
            if KBISECT != 15:
                out += [mk_v(j) for j in range(4)]
            return out

        def emit_qkv(g):
            for f in qkv_pieces(g):
                f()
                if not USE_FP8_DR:
                    nc.vector.tensor_scalar(
                        vb4[:, c, :, 0:32], vps, VSCALE, None, op0=ALU.mult)

        def emit_ffn(gg):
            """h2 block gg -> y block gg -> DMA out."""
            h2t = ps.tile([P, P], BF16, tag="t", name="h2t")
            nc.tensor.transpose(h2t[:], h2_bf[:, ts(gg, P)], idb)
            h2c = work.tile([P, P], BF16, tag="h2c", name="h2c")
            nc.vector.tensor_copy(h2c[:], h2t[:])
            for j in range(4):
                a_ps = ps.tile([P, P], FP32, tag="t", name="a_ps")
                nc.tensor.matmul(
                    a_ps[:], lhsT=w1[ts(j, 32), :], rhs=h2c[ts(j, 32), :],
                    start=True, stop=True, tile_position=(32 * j, 0))
                nc.vector.tensor_scalar(
                    a_sb3[:, 4 * gg + j, :], a_ps[:],
                    fb1, 0.0, op0=ALU.add, op1=ALU.max)
            ff_ps = ps.tile([P, P], FP32, tag="t", name="ff_ps")
            for j in range(4):
                nc.tensor.matmul(
                    ff_ps[ts(j, 32), :], lhsT=w2, rhs=a_sb3[:, 4 * gg + j, :],
                    start=True, stop=True, tile_position=(0, 32 * j))
            ffb = work.tile([P, P], BF16, tag="ffb", name="ffb")
            nc.vector.tensor_scalar(ffb[:], ff_ps[:], b2c, None, op0=ALU.add)
            ftp = ps.tile([P, P], BF16, tag="t", name="ftp")
            nc.tensor.transpose(ftp[:], ffb[:], idb)
            yb = y3[:, 4 * gg : 4 * gg + 4, :]
            nc.vector.tensor_add(yb, ftp.rearrange("p (n d) -> p n d", d=D),
                                 h23[:, 4 * gg : 4 * gg + 4, :])
            nc.sync.dma_start(y_d[:, ts(gg, P)], y_rm[:, ts(gg, P)])

        def bisect_out(src_tile):
            nc.vector.tensor_copy(y_rm[:], src_tile[:])
            nc.sync.dma_start(y_d, y_rm[:])

        # ---- attention + per-g postprocessing, software-pipelined ----
        pend = []
        pend_tail = []
        for g in range(4):
            if g == 1:
                emit_hct([1, 2, 3])
                emit_qkv(1)
                emit_qkv(2)
            elif g == 2:
                emit_qkv(3)
            if g == 0:
                if KBISECT == 11:
                    nc.vector.tensor_copy(y_rm[:], fb[:, _X0 : _X0 + 512])
                    nc.sync.dma_start(y_d, y_rm[:])
                    return
                emit_ln1(0, 4, "lnA")
                if KBISECT == 12:
                    bisect_out(h_rm)
                    return
                emit_hct([0])
                if KBISECT == 13:
                    bisect_out(h_rm)
                    return
                emit_qkv(0)
                if KBISECT in (14, 15, 16):
                    bisect_out(h_rm)
                    return
                if KBISECT == 1:
                    emit_ln1(4, 16, "lnC")
                    bisect_out(h_rm)
                    return

            oA = ps.tile([P, 512], FP32, tag="o", bufs=2, name="oA")
            oB = ps.tile([P, 512], FP32, tag="o", bufs=2, name="oB")
            nch = 4 * g + 4
            p8_live = None
            for c in range(nch):
                m = c - 4 * g
                diag = m >= 0
                lo = 128 * m if diag else 0
                loj = lo // 128
                gc_, jc = divmod(c, 4)
                s0 = ps.tile([P, 2 * 512], FP32, tag="s", bufs=2, name="s0")
                s03 = s0.rearrange("p (h q) -> p h q", h=2)
                s1 = ps.tile([P, 2 * 512], FP32, tag="s", bufs=2, name="s1")
                s13 = s1.rearrange("p (h q) -> p h q", h=2)
                for h in range(4):
                    st = s03 if h < 2 else s13
                    nc.tensor.matmul(
                        st[:, h % 2, lo:],
                        lhsT=kt4[32 * h : 32 * h + HD, jc, gc_, :],
                        rhs=qt4[32 * h : 32 * h + HD, loj:, g, :],
                        start=True, stop=True,
                        tile_position=(32 * h, 0),
                    )
                for fn in pend:
                    fn()
                pend = []
                if diag and m == 2 and KBISECT != 3:
                    mk_half(g, oA, oB, 0)()
                    if g == 3:
                        mk_ln2h(g, 0)()
                if pend_tail:
                    pend_tail.pop(0)()
                if g == 0 and c == 1:
                    # LN1 for groups 1..3: scheduled behind g0's critical
                    # chain (the tile scheduler would otherwise interleave it
                    # into LN1-g0 on VectorE and delay the whole pipeline),
                    # with the big elementwise ops on the idle GpSimd engine.
                    with tc.tile_wait_until(0.009):
                        emit_ln1(4, 16, "lnB", ew=nc.gpsimd)
                if diag:
                    pd = work.tile([P, 4 * 512], BF16, tag="pd", bufs=4, name="pd")
                    pd3 = pd.rearrange("p (h q) -> p h q", h=4)
                    nc.scalar.activation(pd3[:, 0:2, lo:], s03[:, :, lo:], AF.Exp)
                    nc.scalar.activation(pd3[:, 2:4, lo:], s13[:, :, lo:], AF.Exp)
                    nc.vector.tensor_mul(
                        pd3[:, :, lo : lo + P],
                        pd3[:, :, lo : lo + P],
                        msk[:, None, :].to_broadcast((P, 4, P)),
                    )

                    def mk_diag(c=c, lo=lo, pd3=pd3, oA=oA, oB=oB, nch=nch):
                        def f():
                            vsrc = v84 if USE_FP8_DR else vb4
                            for h in range(4):
                                ob = oA if h < 2 else oB
                                base = 64 * (h % 2)
                                nc.tensor.matmul(
                                    ob[base : base + 64, lo:],
                                    lhsT=vsrc[:, c, h, :],
                                    rhs=pd3[:, h, lo:],
                                    start=(c == 0),
                                    stop=(c == nch - 1),
                                    skip_group_check=True,
                                )
                        return f

                    pend.append(mk_diag())
                elif USE_FP8_DR:
                    par = c & 1
                    if par == 0:
                        p8_live = work.tile([P, 2 * 4 * 512], FP8, tag="p8",
                                            bufs=3, name="p8")
                    p84 = p8_live.rearrange("p (r h q) -> p r h q", r=2, h=4)
                    if _dve_pair(g, c // 2):
                        nc.vector.tensor_scalar(
                            p84.bitcast(I8)[:, par, 0:2, :], s03,
                            EXP8_SCALE, EXP8_BIAS, op0=ALU.mult, op1=ALU.add)
                        nc.vector.tensor_scalar(
                            p84.bitcast(I8)[:, par, 2:4, :], s13,
                            EXP8_SCALE, EXP8_BIAS, op0=ALU.mult, op1=ALU.add)
                    else:
                        nc.scalar.activation(p84[:, par, 0:2, :], s03, AF.Exp)
                        nc.scalar.activation(p84[:, par, 2:4, :], s13, AF.Exp)
                    if par == 1:
                        def mk_pair(c=c, p84=p84, oA=oA, oB=oB):
                            def f():
                                # even heads: DoubleRow fp8 (tile position 0
                                # only -- walrus rejects DR at col offset 64);
                                # odd heads: plain fp8 matmuls per chunk.
                                for h in (0, 2):
                                    ob = oA if h < 2 else oB
                                    for qh in range(2):
                                        nc.tensor.matmul(
                                            ob[0:64, ts(qh, 256)],
                                            lhsT=v84[:, c - 1 : c + 1, h, :],
                                            rhs=p84[:, :, h, ts(qh, 256)],
                                            perf_mode=DR,
                                            start=(c == 1),
                                            stop=False,
                                            skip_group_check=True,
                                        )
                                for h in (1, 3):
                                    ob = oA if h < 2 else oB
                                    for cc in (c - 1, c):
                                        nc.tensor.matmul(
                                            ob[64:128, :],
                                            lhsT=v84[:, cc, h, :],
                                            rhs=p84[:, cc & 1, h, :],
                                            start=(cc == 0),
                                            stop=False,
                                            tile_position=(0, 64),
                                            skip_group_check=True,
                                        )
                            return f

                        pend.append(mk_pair())
                else:
                    pdf = work.tile([P, 4 * 512], BF16, tag="pd", bufs=4, name="pdf")
                    pdf3 = pdf.rearrange("p (h q) -> p h q", h=4)
                    nc.scalar.activation(pdf3[:, 0:2, :], s03, AF.Exp)
                    nc.scalar.activation(pdf3[:, 2:4, :], s13, AF.Exp)

                    def mk_full(c=c, pdf3=pdf3, oA=oA, oB=oB):
                        def f():
                            for h in range(4):
                                ob = oA if h < 2 else oB
                                base = 64 * (h % 2)
                                nc.tensor.matmul(
                                    ob[base : base + 64, :],
                                    lhsT=vb4[:, c, h, :],
                                    rhs=pdf3[:, h, :],
                                    start=(c == 0),
                                    stop=False,
                                    skip_group_check=True,
                                )
                        return f

                    pend.append(mk_full())
            for fn in pend:
                fn()
            pend = []

            def mk_half(g, oA, oB, half):
                def f():
                    # finalize q'-columns [256*half, 256*half+256) of O':
                    # transpose back, divide by den, sum heads, residual.
                    osbA = work.tile([P, 256], BF16, tag="osb", bufs=4,
                                     name="osbA")
                    nc.vector.tensor_copy(osbA[:], oA[:, ts(half, 256)])
                    osbB = work.tile([P, 256], BF16, tag="osb", bufs=4,
                                     name="osbB")
                    nc.vector.tensor_copy(osbB[:], oB[:, ts(half, 256)])
                    otpA = ps.tile([P, 256], BF16, tag="t", name="otpA")
                    otpB = ps.tile([P, 256], BF16, tag="t", name="otpB")
                    for jj in range(2):
                        nc.tensor.transpose(otpA[:, ts(jj, P)],
                                            osbA[:, ts(jj, P)], idb)
                    for jj in range(2):
                        nc.tensor.transpose(otpB[:, ts(jj, P)],
                                            osbB[:, ts(jj, P)], idb)
                    oa4 = otpA.rearrange("p (j k e) -> p j k e", j=2, k=2)
                    ob4 = otpB.rearrange("p (j k e) -> p j k e", j=2, k=2)
                    drA = work.tile([P, 2 * 2], FP32, tag="dr", name="drA")
                    drA3 = drA.rearrange("p (j k) -> p j k", j=2)
                    nc.vector.reciprocal(drA3, oa4[:, :, :, 32])
                    drB = work.tile([P, 2 * 2], FP32, tag="dr", name="drB")
                    drB3 = drB.rearrange("p (j k) -> p j k", j=2)
                    nc.vector.reciprocal(drB3, ob4[:, :, :, 32])
                    tmA = work.tile([P, 2 * 2 * 32], FP32, tag="tm", name="tmA")
                    tmA4 = tmA.rearrange("p (j k e) -> p j k e", j=2, k=2)
                    nc.vector.tensor_mul(
                        tmA4, oa4[:, :, :, 0:32],
                        drA3[:, :, :, None].to_broadcast((P, 2, 2, 32)))
                    tmB = work.tile([P, 2 * 2 * 32], FP32, tag="tm", name="tmB")
                    tmB4 = tmB.rearrange("p (j k e) -> p j k e", j=2, k=2)
                    nc.vector.tensor_mul(
                        tmB4, ob4[:, :, :, 0:32],
                        drB3[:, :, :, None].to_broadcast((P, 2, 2, 32)))
                    n0 = 4 * g + 2 * half
                    u1 = work.tile([P, 2 * 32], FP32, tag="u", name="u1")
                    u13 = u1.rearrange("p (j e) -> p j e", j=2)
                    nc.vector.tensor_add(u13, tmA4[:, :, 0, :], tmA4[:, :, 1, :])
                    u2 = work.tile([P, 2 * 32], FP32, tag="u", name="u2")
                    u23 = u2.rearrange("p (j e) -> p j e", j=2)
                    nc.vector.tensor_add(u23, tmB4[:, :, 0, :], tmB4[:, :, 1, :])
                    nc.vector.tensor_add(u13, u13, u23)
                    nc.vector.tensor_add(x13[:, n0 : n0 + 2, :], u13,
                                         hb3[:, n0 : n0 + 2, :])
                return f

            def mk_ln2(g):
                def f():
                    layer_norm(x13[:, 4 * g : 4 * g + 4, :], g2, lb2,
                               h23[:, 4 * g : 4 * g + 4, :], 4, "ln2")
                    nc.vector.tensor_copy(h2b3[:, 4 * g : 4 * g + 4, :],
                                          h23[:, 4 * g : 4 * g + 4, :])
                return f

            def mk_ln2h(g, half):
                def f():
                    n0 = 4 * g + 2 * half
                    layer_norm(x13[:, n0 : n0 + 2, :], g2, lb2,
                               h23[:, n0 : n0 + 2, :], 2, "ln2")
                    nc.vector.tensor_copy(h2b3[:, n0 : n0 + 2, :],
                                          h23[:, n0 : n0 + 2, :])
                return f

            if KBISECT == 3:
                pend_tail = []
            elif KBISECT == 4:
                pend_tail = [mk_half(g, oA, oB, 1)]
                if g == 3:
                    pend_tail[0]()
                    pend_tail = []
            elif g == 3:
                mk_half(g, oA, oB, 1)()
                mk_ln2h(g, 1)()
                emit_ffn(g)
                pend_tail = []
            else:
                pend_tail = [mk_half(g, oA, oB, 1), mk_ln2(g),
                             lambda g=g: emit_ffn(g)]


        if KBISECT == 2:
            bisect_out(hb)
        elif KBISECT == 3:
            bisect_out(hb)
        elif KBISECT == 4:
            bisect_out(x1_rm)


def _host_blobs(inputs):
    Wq = np.asarray(inputs["Wq"], np.float32)
    Wk = np.asarray(inputs["Wk"], np.float32)
    Wv = np.asarray(inputs["Wv"], np.float32)
    Wproj = np.asarray(inputs["Wproj"], np.float32)
    scale = float(HD) ** -0.5

    def pad_heads(W):  # [H, D, HD] -> [32, 128] block layout [d, 32h+hd]
        out = np.zeros((D, P), np.float32)
        for h in range(H):
            out[:, 32 * h : 32 * h + HD] = W[h]
        return out

    wq_pad = np.tile(pad_heads(Wq * scale), (4, 1))
    wk_pad = np.tile(pad_heads(Wk), (4, 1))
    wvp = np.zeros((D, P), np.float32)
    for h in range(H):
        wvp[:, 32 * h : 32 * h + 32] = Wv[h] @ Wproj[HD * h : HD * h + HD]
    wvp = np.tile(wvp, (4, 1))

    bblob = np.zeros((P, NBF), np.float32)
    bblob[:, _WQ : _WQ + P] = wq_pad
    bblob[:, _WK : _WK + P] = wk_pad
    bblob[:, _WVP : _WVP + P] = wvp
    bblob[:, _W1 : _W1 + P] = np.tile(np.asarray(inputs["W1"], np.float32), (4, 1))
    bblob[:, _W2 : _W2 + D] = np.asarray(inputs["W2"], np.float32)
    bblob[:, _IDB : _IDB + P] = np.eye(P, dtype=np.float32)
    bblob[:, _MSK : _MSK + P] = np.triu(np.ones((P, P), np.float32))
    bblob = bblob.astype(ml_dtypes.bfloat16)

    def rep(name):
        return np.tile(np.asarray(inputs[name], np.float32)[None, :], (P, 1))

    fblob = np.zeros((P, NF32), np.float32)
    fblob[:, _G1 : _G1 + D] = rep("ln1_g")
    fblob[:, _B1 : _B1 + D] = rep("ln1_b")
    fblob[:, _G2 : _G2 + D] = rep("ln2_g")
    fblob[:, _LB2 : _LB2 + D] = rep("ln2_b")
    fblob[:, _BPJ : _BPJ + D] = rep("bproj")
    fblob[:, _B2R : _B2R + D] = rep("b2")
    fblob[:, _FB1 : _FB1 + 1] = np.asarray(inputs["b1"], np.float32).reshape(FF, 1)
    fblob[:, _B2C : _B2C + 1] = np.tile(np.asarray(inputs["b2"], np.float32), 4).reshape(P, 1)
    return fblob, bblob


def _get_nc():
    if "nc" not in _NC_CACHE:
        _NC_CACHE["nc"] = _build_nc()
    return _NC_CACHE["nc"]


def _run(inputs, trace=False):
    x = np.asarray(inputs["x"], np.float32)
    fblob, bblob = _host_blobs(inputs)
    nc = _get_nc()
    in_maps = []
    for b in range(B):
        fbm = fblob.copy()
        # x_rm[p, n*32+d] = x[b, 128n+p, d]
        fbm[:, _X0 : _X0 + 512] = (
            x[b].reshape(16, P, D).transpose(1, 0, 2).reshape(P, 512)
        )
        in_maps.append({"fblob": np.ascontiguousarray(fbm), "bblob": bblob})
    res = run_bass_kernel_spmd(nc, in_maps, core_ids=list(range(NCORES)), trace=trace)
    outs = []
    for r in res.results:
        yb = r["y"].astype(np.float32)
        outs.append(yb.reshape(P, 16, D).transpose(1, 0, 2).reshape(T, D))
    return np.stack(outs, axis=0), res


def kernel(**inputs):
    out, _ = _run(inputs)
    return out


def kernel_traced(**inputs):
    out, res = _run(inputs, trace=True)
    return out, res
